# revision 1
# baseline (speedup 1.0000x reference)
"""Causal self-attention (RoPE) on 8 trn2 NeuronCores.

Sharding: tensor-parallel over heads; each core owns 2 of 16 heads.
Host sums the 8 partial projection outputs (the all-reduce) + bproj.

v2 design (vs baseline):
 - qkv^T computed directly: stationary = Wqkv 128x128 blocks, moving =
   x^T chunks -> psum [n=128, t=512]. q,k come out already in [d, t]
   layout (no PE transposes); v^T is PE-transposed back to [t, d].
 - RoPE applied on the qkv^T psum via partition-shifted tensor ops
   (in0=PSUM may sit at a different base partition than out/in1).
 - Softmax denominator: ones-column trick -> row 64 of po; reciprocal
   on the row, then a PE broadcast-matmul (ones[1,64].T @ row[1,q])
   builds the per-head [64, q] normalizer, fused into the y^T drain
   multiply. No HBM round-trips, no DMA broadcasts.
 - y^T assembled by partition-shifted psum->SBUF engine ops.
 - Few, large DMAs (each dispatch costs ~0.6us on its engine).
 - Projection interleaved as PE filler inside the attention j-loops.
"""

import ml_dtypes
import numpy as np

import concourse.bacc as bacc
import concourse.bass as bass
import concourse.mybir as mybir
import concourse.tile as tile
from concourse.bass_utils import run_bass_kernel_spmd

F32 = mybir.dt.float32
F32R = mybir.dt.float32r
FP16 = mybir.dt.float16
BF16 = mybir.dt.bfloat16

B, T, C = 2, 2048, 1024
H, D = 16, 64
NCORES = 8
HL = 2                   # heads per core
R = B * T                # 4096 token rows
PB = 128
TBB = T // PB            # 16 row blocks per batch
QT = 512                 # attention query tile
NQT = T // QT            # 4 per batch
KC = C // PB             # 8 contraction chunks
NW = R // QT             # 8 qkv waves (one 512-token chunk each)
ROPE_BASE = 10000.0

MM_DT = BF16


def _build_nc(with_bias=False):
    nc = bacc.Bacc(trn_type="TRN2")

    xT = nc.dram_tensor("xT", [C, R], MM_DT, kind="ExternalInput")
    wq = nc.dram_tensor("wq", [C, 3 * HL * D], MM_DT, kind="ExternalInput")
    wp = nc.dram_tensor("wp", [HL * D, C], MM_DT, kind="ExternalInput")
    tbl = nc.dram_tensor("tbl", [PB, 2, T], MM_DT, kind="ExternalInput")
    idn = nc.dram_tensor("idn", [PB, PB], MM_DT, kind="ExternalInput")
    bm2 = nc.dram_tensor("bm2", [PB, 2 * PB], MM_DT, kind="ExternalInput")
    out = nc.dram_tensor("out", [R, C], MM_DT, kind="ExternalOutput")
    if with_bias:
        btbl = nc.dram_tensor("btbl", [PB, 2, T], F32, kind="ExternalInput")
        bv = nc.dram_tensor("bv", [PB, 1], F32, kind="ExternalInput")
    else:
        btbl = bv = None

    with tile.TileContext(nc) as tc:
        _body(nc, tc, xT, wq, wp, tbl, idn, bm2, out, btbl, bv)
    nc.finalize()
    return nc


def _body(nc, tc, xT, wq, wp, tbl, idn, bm2, out, btbl, bv):
    import contextlib

    ctx = contextlib.ExitStack()
    with ctx:
        singles = ctx.enter_context(tc.tile_pool(name="singles", bufs=1))

        # ---- resident constants -------------------------------------------
        # wq in 3 pieces (kc 0:2, 2:5, 5:8) so the first matmuls can start
        # as soon as the first slices land; dispatched on the scalar HW-DGE
        # queue in parallel with x chunks on sync.
        KCB = [(0, 2), (2, 5), (5, 8)]
        wq_r = wq.rearrange("(kc p) n -> p kc n", p=PB)
        wq_p = []
        for lo, hi in KCB:
            t_ = singles.tile([PB, hi - lo, 3 * PB], MM_DT, name=f"wq{lo}")
            nc.scalar.dma_start(out=t_, in_=wq_r[:, lo:hi, :])
            wq_p.append(t_)

        def wq_at(kc, ncols):
            i = 0 if kc < 2 else (1 if kc < 5 else 2)
            return wq_p[i][:, kc - KCB[i][0], ncols]

        tbl_t = singles.tile([PB, 2, T], MM_DT)
        nc.gpsimd.dma_start(out=tbl_t, in_=tbl[:, :, :])
        idn_t = singles.tile([PB, PB], MM_DT)
        nc.gpsimd.dma_start(out=idn_t, in_=idn[:, :])
        bm_t = singles.tile([PB, 2, PB], MM_DT)
        wp_t = singles.tile([PB, C], MM_DT)
        if btbl is not None:
            btbl_t = singles.tile([PB, 2, T], F32)
            nc.gpsimd.dma_start(out=btbl_t, in_=btbl[:, :, :])
            bv_t = singles.tile([PB, 1], F32)
            nc.gpsimd.dma_start(out=bv_t, in_=bv[:, :])

        ones_f = singles.tile([PB, 64], F32)
        nc.vector.memset(ones_f, 1.0)
        dz = singles.tile([PB, QT], MM_DT, name="dz")
        nc.gpsimd.memset(dz, 0.0)
        idn_h = singles.tile([PB, PB], FP16)
        nc.vector.tensor_copy(idn_h, idn_t)

        # ---- resident activations -----------------------------------------
        qkT_b = [
            singles.tile([PB, TBB, 2, PB], MM_DT, name=f"qkT{b}") for b in range(B)
        ]
        va_b = [
            singles.tile([PB, HL, TBB, PB], MM_DT, name=f"va{b}")
            for b in range(B)
        ]
        yT = singles.tile([PB, R], MM_DT)

        for b in range(B):
            nc.gpsimd.memset(va_b[b], 1.0)

        with (
            tc.tile_pool(name="xt", bufs=4) as xt_pool,
            tc.tile_pool(name="qn", bufs=2) as qn_pool,
            tc.tile_pool(name="dsb", bufs=2) as dsb_pool,
            tc.tile_pool(name="pt", bufs=8) as pt_pool,
            tc.tile_pool(name="ost", bufs=6) as ost_pool,
            tc.tile_pool(name="psq", bufs=2, space="PSUM") as psq_pool,
            tc.tile_pool(name="pss", bufs=2, space="PSUM") as pss_pool,
            tc.tile_pool(name="pso", bufs=1, space="PSUM") as pso_pool,
        ):
            xtt = {}

            xT_r = xT.rearrange("(kc p) t -> p kc t", p=PB)

            def prefetch(w, split=False):
                if w >= NW or w in xtt:
                    return
                cs = slice(w * QT, (w + 1) * QT)
                if split:
                    ps = []
                    for gi, (lo, hi) in enumerate(KCB):
                        t_ = xt_pool.tile([PB, hi - lo, QT], MM_DT,
                                          tag=f"xts{gi}", name=f"xt{w}_{gi}")
                        nc.sync.dma_start(out=t_, in_=xT_r[:, lo:hi, cs])
                        ps.append(t_)
                    xtt[w] = ps
                else:
                    t_ = xt_pool.tile([PB, KC, QT], MM_DT, tag="xt",
                                      name=f"xt{w}")
                    nc.sync.dma_start(out=t_, in_=xT_r[:, :, cs])
                    xtt[w] = t_

            def xt_at(w, kc):
                v = xtt[w]
                if isinstance(v, list):
                    i = 0 if kc < 2 else (1 if kc < 5 else 2)
                    return v[i][:, kc - KCB[i][0], :]
                return v[:, kc, :]

            # ---------------- qkv^T wave (one 512-token chunk) -------------
            def wave_nblk(w, nblk):
                if nblk == 0:
                    prefetch(w + 3)
                b, tc4 = divmod(w, NQT)
                tb0 = tc4 * 4
                cols = slice(tc4 * QT, (tc4 + 1) * QT)   # within-batch t
                if True:
                    psq = psq_pool.tile([PB, QT], F32, tag="psq", name="psq")
                    ncols = slice(nblk * PB, (nblk + 1) * PB)
                    for kc in range(KC):
                        nc.tensor.matmul(
                            psq,
                            lhsT=wq_at(kc, ncols),
                            rhs=xt_at(w, kc),
                            start=(kc == 0),
                            stop=(kc == KC - 1),
                        )
                    if nblk < 2:
                        # RoPE: rt = shifted(psq) * sgn ; qc = psq * cos ;
                        # qkT slice = qc + rt  (+ btbl if biased)
                        rt = qn_pool.tile([PB, QT], MM_DT, tag="rt", name="rt")
                        for qd in range(4):
                            ob, ib = qd * 32, (qd ^ 1) * 32
                            nc.vector.tensor_tensor(
                                out=rt[ob : ob + 32, :],
                                in0=psq[ib : ib + 32, :],
                                in1=tbl_t[ob : ob + 32, 1, cols],
                                op=mybir.AluOpType.mult,
                            )
                        qc = qn_pool.tile([PB, QT], MM_DT, tag="qc", name="qc")
                        nc.vector.tensor_tensor(
                            out=qc, in0=psq, in1=tbl_t[:, 0, cols],
                            op=mybir.AluOpType.mult,
                        )
                        dst = qkT_b[b][:, tb0 : tb0 + 4, nblk, :]
                        if btbl is None:
                            nc.gpsimd.tensor_tensor(
                                out=dst,
                                in0=qc.rearrange("p (a b) -> p a b", a=4),
                                in1=rt.rearrange("p (a b) -> p a b", a=4),
                                op=mybir.AluOpType.add,
                            )
                        else:
                            qr = qn_pool.tile([PB, QT], F32, tag="qr", name="qr")
                            nc.vector.tensor_tensor(
                                out=qr, in0=qc, in1=rt, op=mybir.AluOpType.add,
                            )
                            nc.vector.tensor_tensor(
                                out=dst,
                                in0=qr.rearrange("p (a b) -> p a b", a=4),
                                in1=btbl_t[:, nblk, cols].rearrange(
                                    "p (a b) -> p a b", a=4
                                ),
                                op=mybir.AluOpType.add,
                            )
                    else:
                        vn = qn_pool.tile([PB, QT], MM_DT, tag="vn", name="vn")
                        if btbl is None:
                            nc.scalar.copy(vn, psq)
                        else:
                            nc.scalar.add(vn, psq, bv_t[:, 0:1])
                        vtr = psq_pool.tile(
                            [PB, 4, PB], MM_DT, tag="psq", name="vtr"
                        )
                        for i in range(4):
                            nc.tensor.transpose(
                                vtr[:, i, :], vn[:, i * PB : (i + 1) * PB], idn_t
                            )
                        nc.scalar.copy(
                            va_b[b][:, :, tb0 : tb0 + 4, 0:64],
                            vtr.rearrange("p i (h d) -> p h i d", h=2),
                        )

            # ---------------- filler machinery ------------------------------
            # waveq: (wave_id, closure) units; drainq: denominator-chain
            # closures; pending: proj row-blocks. One unit is emitted per
            # attention j-iteration to keep the PE queue dense.
            waveq = []
            drainq = []
            pending = []

            def emit_one_proj():
                if not pending:
                    return False
                qb = pending.pop(0)
                ot = ost_pool.tile([PB, C], MM_DT, tag="ot", name="ot")
                for nch in range(2):
                    pp = psq_pool.tile([PB, QT], F32, tag="psq", name="pp")
                    nc.tensor.matmul(
                        pp,
                        lhsT=yT[:, qb * PB : (qb + 1) * PB],
                        rhs=wp_t[:, nch * QT : (nch + 1) * QT],
                        start=True,
                        stop=True,
                    )
                    if nch == 0:
                        nc.vector.tensor_copy(ot[:, 0:QT], pp)
                    else:
                        nc.scalar.copy(ot[:, QT:C], pp)
                nc.sync.dma_start(out=out[qb * PB : (qb + 1) * PB, :], in_=ot)
                return True

            cur_po = []

            def emit_fill(allow_proj=True):
                if drainq:
                    drainq.pop(0)()
                elif waveq:
                    waveq.pop(0)[1]()
                elif allow_proj and emit_one_proj():
                    pass
                elif cur_po:
                    # keep the PE clock-gate warm: a throwaway matmul into
                    # the never-read rows 96:128 of the live po tile.
                    nc.tensor.matmul(
                        cur_po[0][96:128, :],
                        lhsT=dz[:, 0:32],
                        rhs=dz,
                        start=True,
                        stop=True,
                        tile_position=(0, 96),
                        skip_group_check=True,
                    )

            def queue_wave(w):
                for nblk in range(3):
                    waveq.append((w, lambda w=w, n=nblk: wave_nblk(w, n)))

            def flush_waves(k):
                while waveq and waveq[0][0] <= k:
                    waveq.pop(0)[1]()

            # ---------------- attention for one query tile -----------------
            def attn_qt(b, qt, fill=0):
                po = [
                    pso_pool.tile([PB, QT], F32, tag=f"po{h}", name=f"po{h}")
                    for h in range(HL)
                ]
                cur_po.clear()
                cur_po.append(po[0])
                jmax = qt * 4 + 4

                def s_off(j):
                    return max(j - qt * 4, 0) * PB

                def emit_scores(j):
                    off = s_off(j)
                    ps = pss_pool.tile([PB, HL, QT], F32, tag="pss", name="ps")
                    for h in range(HL):
                        nc.tensor.matmul(
                            ps[:, h, off:QT],
                            lhsT=qkT_b[b][h * 64 : h * 64 + 64, j, 1, :],
                            rhs=qkT_b[b][
                                h * 64 : h * 64 + 64,
                                qt * 4 + off // PB : qt * 4 + 4, 0, :,
                            ],
                            start=True,
                            stop=True,
                        )
                    return ps

                ps_cur = emit_scores(0)
                for j in range(jmax):
                    m = j - qt * 4
                    off = s_off(j)
                    pt = pt_pool.tile([PB, HL, QT], MM_DT, tag="pt", name="pt")
                    nc.scalar.activation(
                        out=pt[:, :, off:QT], in_=ps_cur[:, :, off:QT],
                        func=mybir.ActivationFunctionType.Exp, scale=0.125,
                    )
                    if m >= 0:
                        nc.gpsimd.tensor_tensor(
                            out=pt[:, :, off : off + PB],
                            in0=pt[:, :, off : off + PB],
                            in1=bm_t, op=mybir.AluOpType.mult,
                        )
                    if j + 1 < jmax:
                        ps_nxt = emit_scores(j + 1)
                    emit_fill()
                    for h in range(HL):
                        nc.tensor.matmul(
                            po[h][:, off:QT],
                            lhsT=va_b[b][:, h, j, :],
                            rhs=pt[:, h, off:QT],
                            start=(j == 0),
                            stop=(j == jmax - 1),
                        )
                    if j + 1 < jmax:
                        ps_cur = ps_nxt

                # ---- release po fast: den row copies + raw y^T copies -----
                cols = slice(b * T + qt * QT, b * T + (qt + 1) * QT)
                den = dsb_pool.tile([65, HL, QT], F32, tag="den", name="den")
                nc.scalar.copy(den[64:65, 0, :], po[0][64:65, :])
                nc.vector.tensor_copy(den[64:65, 1, :], po[1][64:65, :])
                yraw = dsb_pool.tile([PB, QT], MM_DT, tag="yraw", name="yraw")
                nc.vector.tensor_copy(yraw[0:64, :], po[0][0:64, :])
                nc.vector.tensor_copy(yraw[64:128, :], po[1][0:64, :])

                def drain1(den=den):
                    # den rows -> columns (PE), tiny parallel reciprocal,
                    # replicate reciprocal columns for the broadcast matmul.
                    dtr = psq_pool.tile([PB, HL, 4], F32, tag="psq", name="dtr")
                    for h in range(HL):
                        for i in range(4):
                            nc.tensor.matmul(
                                dtr[:, h, i : i + 1],
                                lhsT=den[64:65, h, i * PB : (i + 1) * PB],
                                rhs=ones_f[64:65, 0:1],
                                start=True,
                                stop=True,
                                tile_position=(64, 0),
                            )
                    rec = dsb_pool.tile([PB, HL, 4], FP16, tag="rec",
                                        name="rec")
                    with nc.allow_low_precision(reason="fp16 recip of den"):
                        nc.vector.reciprocal(rec, dtr)
                    rec2 = dsb_pool.tile([PB, HL, 4, 64], FP16, tag="rec2",
                                         name="rec2")
                    nc.vector.tensor_copy(
                        rec2,
                        bass.AP(tensor=rec.tensor, offset=rec.offset,
                                ap=[list(rec.ap[0]), [4, 2], [1, 4], [0, 64]]),
                    )
                    for i in range(4):
                        drainq.append(lambda i=i: drain2(rec2, i))

                def drain2(rec2, i, b=b, qt=qt, yraw=yraw):
                    # broadcast per-head reciprocal and normalize one
                    # 128-query block of y^T, then queue its projection.
                    qb = (b * T + qt * QT) // PB + i
                    rn = psq_pool.tile([PB, PB], F32, tag="psq", name="rn")
                    for h in range(HL):
                        nc.tensor.matmul(
                            rn[h * 64 : h * 64 + 64, :],
                            lhsT=rec2[:, h, i, :],
                            rhs=idn_h,
                            start=True,
                            stop=True,
                        )
                    rn_sb = dsb_pool.tile([PB, PB], FP16, tag=f"rn{i % 2}",
                                          name="rn_sb")
                    nc.vector.tensor_copy(rn_sb, rn)
                    nc.gpsimd.tensor_tensor(
                        out=yT[:, qb * PB : (qb + 1) * PB],
                        in0=yraw[:, i * PB : (i + 1) * PB], in1=rn_sb,
                        op=mybir.AluOpType.mult,
                    )
                    pending.append(qb)

                drainq.append(lambda: drain1())

            # ---------------- schedule -------------------------------------
            prefetch(0, split=True)
            prefetch(1, split=True)
            prefetch(2)
            # PE warmup: dummy matmuls during the DMA-init window keep the
            # HAM clock-gate at full rate for the first real wave.
            wz = qn_pool.tile([PB, PB], MM_DT, tag="rt", name="warm")
            nc.vector.memset(wz, 0.0)
            pw = psq_pool.tile([PB, QT], F32, tag="psq", name="pw")
            for i in range(128):
                nc.tensor.matmul(pw[:, 0:PB], lhsT=wz, rhs=wz,
                                 start=True, stop=True)
            nc.gpsimd.dma_start(out=bm_t,
                                in_=bm2.rearrange("p (g x) -> p g x", g=2))
            nc.gpsimd.dma_start(out=wp_t, in_=wp[:, :])
            for w in range(NW):
                queue_wave(w)
            for b in range(B):
                for qt in range(NQT):
                    flush_waves(b * NQT + qt)
                    attn_qt(b, qt)
            while drainq or waveq or pending:
                if waveq:
                    waveq.pop(0)[1]()
                if drainq:
                    drainq.pop(0)()
                emit_one_proj()


_NC_CACHE = {}
LAST_RESULTS = None


def _tables():
    inv = 1.0 / (ROPE_BASE ** (np.arange(0, D, 2, dtype=np.float32) / D))
    t = np.arange(T, dtype=np.float32)
    fr = np.einsum("i,j->ij", t, inv)             # [T, 32]
    emb = np.concatenate([fr, fr], axis=1)        # [T, 64]
    cos = np.cos(emb).T.astype(np.float32)        # [64, T]
    sin = np.sin(emb).T.astype(np.float32)
    sgn = np.concatenate([-sin[0:32], sin[32:64]], axis=0)   # [64, T]
    cos2 = np.concatenate([cos, cos], axis=0)     # [128, T] two heads
    sgn2 = np.concatenate([sgn, sgn], axis=0)
    tbl = np.stack([cos2, sgn2], axis=1)          # [128, 2, T]
    return np.ascontiguousarray(tbl).astype(ml_dtypes.bfloat16)


def kernel(x, Wqkv, bqkv, Wproj, bproj):
    global LAST_RESULTS
    x = np.asarray(x, dtype=np.float32)
    Wqkv = np.asarray(Wqkv, dtype=np.float32)
    bqkv = np.asarray(bqkv, dtype=np.float32)
    Wproj = np.asarray(Wproj, dtype=np.float32)
    bproj = np.asarray(bproj, dtype=np.float32)
    with_bias = bool(np.any(bqkv))

    xT = np.ascontiguousarray(x.reshape(R, C).T).astype(ml_dtypes.bfloat16)
    tbl = _tables()
    idn = np.eye(PB, dtype=np.float32).astype(ml_dtypes.bfloat16)
    bm0 = (np.tril(np.ones((PB, PB), dtype=np.float32))).T  # [k,u]: u>=k
    bm2 = np.ascontiguousarray(np.concatenate([bm0, bm0], axis=1)).astype(
        ml_dtypes.bfloat16
    )

    in_maps = []
    for r in range(NCORES):
        hsel = [2 * r, 2 * r + 1]
        wcols = []
        for part in range(3):  # q, k, v column groups
            for h in hsel:
                wcols.append(Wqkv[:, part * C + h * D : part * C + (h + 1) * D])
        wq_l = np.ascontiguousarray(np.concatenate(wcols, axis=1)).astype(
            ml_dtypes.bfloat16
        )
        wp_l = np.ascontiguousarray(Wproj[r * PB : (r + 1) * PB, :]).astype(
            ml_dtypes.bfloat16
        )
        m = {"xT": xT, "wq": wq_l, "wp": wp_l, "tbl": tbl, "idn": idn,
             "bm2": bm2}
        if with_bias:
            bq_cols = []
            for part in range(3):
                for h in hsel:
                    bq_cols.append(
                        bqkv[part * C + h * D : part * C + (h + 1) * D]
                    )
            bq_l = np.stack([np.concatenate(bq_cols[0:2]),
                             np.concatenate(bq_cols[2:4])])   # [2, 128] q,k
            bvv = np.concatenate(bq_cols[4:6]).reshape(PB, 1)  # [128,1] v

            def rot(vec):
                o = np.empty_like(vec)
                for base in (0, 64):
                    o[base:base + 32] = -vec[base + 32:base + 64]
                    o[base + 32:base + 64] = vec[base:base + 32]
                return o

            inv = 1.0 / (ROPE_BASE ** (np.arange(0, D, 2) / D))
            tt = np.arange(T, dtype=np.float32)
            fr = np.einsum("i,j->ij", tt, inv)
            emb = np.concatenate([fr, fr], axis=1)
            cosf = np.concatenate([np.cos(emb).T] * 2, axis=0).astype(np.float32)
            sinf = np.concatenate([np.sin(emb).T] * 2, axis=0).astype(np.float32)
            bt = np.empty((PB, 2, T), np.float32)
            for g in range(2):
                bcol = bq_l[g][:, None].astype(np.float32)
                bt[:, g, :] = bcol * cosf + rot(bcol[:, 0])[:, None] * sinf
            m["btbl"] = bt
            m["bv"] = np.ascontiguousarray(bvv.astype(np.float32))
        in_maps.append(m)

    key = with_bias
    if key not in _NC_CACHE:
        _NC_CACHE[key] = _build_nc(with_bias)
    res = run_bass_kernel_spmd(_NC_CACHE[key], in_maps,
                               core_ids=list(range(NCORES)))
    LAST_RESULTS = res
    acc = np.zeros((R, C), dtype=np.float32)
    for r in range(NCORES):
        acc += res.results[r]["out"].astype(np.float32)
    acc += bproj[None, :]
    return acc.reshape(B, T, C)

